# revision 55
# baseline (speedup 1.0000x reference)
"""Trainium2 Bass kernel for nn_CrossAttentionLayer (ragged cross-attention).

Sharding: data-parallel over the 16 ragged samples -> 2 samples per core
(8 cores). Weights replicated (host-packed per layout below).

Device pipeline per 256-token iteration (per core, per sample slot):
  - DMA one fp8e4m3 kv chunk [128, 2, 256]
  - K-proj / V-proj as single fp8 DoubleRow matmuls (K=256 in one
    instruction); weights pre-scaled x16 on host for fp8 dynamic range
  - kT and v copies PSUM->SBUF on DVE (bf16 / fp8)
  - scores in bf16 (block-diag 4-head packing); exp on the Act engine
    with the mask plus a -3ln2 bias fused, output in fp8e5m2 (overflow-
    safe range for e^s, and p errors average out over 4096 tokens)
  - context accumulated in the out[q, d] orientation via fp8 DoubleRow
    over block pairs; softmax denominator via per-head N=1 matmuls into
    the same PSUM bank (one accumulation group per bank zero-region)
All biases are folded on the host: bq into the pre-projected q-tilde,
bk vanishes (softmax shift invariance), bv/bo into the residual term.
Finalize (deferred into the next slot's pipeline): reciprocal +
broadcast scale (1/l / 16; the exp bias cancels in ctx/l), PE
transpose, out-projection, residual add, per-slot output DMA.
Schedule: software-pipelined so the in-order PE stream never stalls on
exp; the Act engine is the bottleneck (~56us of exp) and runs back-to-
back; first chunks' kT are host-injected to shorten warmup; the exp
activation table is preloaded by a dummy exp at t=0.
"""
import sys
import numpy as np

sys.path.insert(0, "/opt/trn_rl_repo")

import ml_dtypes  # noqa: E402

BF16 = ml_dtypes.bfloat16
FP8 = ml_dtypes.float8_e4m3

D = 256
H = 8
HD = 32
NQ = 100
NCORES = 8
S = 2  # sample slots per core
WSCALE = 16.0  # fp8 weight pre-scale (k and v paths)
EXPBIAS = -3.0 * float(np.log(2.0))  # keep e^s in fp8 range

_prog_cache = {}
TRACE_SIM = False


def _ceil_to(x, m):
    return ((x + m - 1) // m) * m


def _patch_tile_drain():
    """walrus CoreV3 CTRL codegen rejects >2 sem-waits on one Drain; the
    Tile kernel-tail drain aggregates one wait per live proc. Split the
    waits across preceding single-wait SP nops instead."""
    from concourse import mybir
    from concourse import tile as tile_mod

    if getattr(tile_mod.TileContext, "_drain_patched", False):
        return

    def _drain_and_barrier(self, tick_clock, wait_clock):
        nc = self.nc
        carrier = nc.sync.nop(nofuse=True)
        wait_clock.add_sem_waits(
            carrier.ins, tile_mod.ScopedClock({None: tick_clock.global_clock}))
        si = carrier.ins.sync_info
        waits = list(si.on_wait) if si and si.on_wait else []
        MAXW = 1
        if len(waits) > MAXW:
            si.on_wait = waits[:MAXW]
            for i in range(MAXW, len(waits), MAXW):
                nop = nc.sync.nop(nofuse=True)
                nop.ins.sync_info = mybir.SyncInfo(
                    on_wait=waits[i:i + MAXW], on_update=[])
        nc.sync.drain()
        nc.all_engine_barrier()
        popped = nc._tile_sem_poison_stack.pop()
        assert popped is self._sem_poison
        nc.clear_and_free_semaphores(list(self.sems.allocated().values()))
        nc.all_engine_barrier()

    tile_mod.TileContext._drain_and_barrier = _drain_and_barrier
    tile_mod.TileContext._drain_patched = True


def _split_bir_waits(m, maxw=1):
    """walrus CoreV2/V3 codegen rejects instructions carrying more than one
    sync-wait command. Hoist extra waits onto same-engine NoOps inserted
    immediately before the instruction (engine execution is in-order, so
    the happens-before is preserved)."""
    uid = [0]
    for fn in m.get("functions", []):
        for bb in fn.get("blocks", []):
            out = []
            for ins in bb.get("instructions", []):
                si = ins.get("sync_info")
                waits = (si or {}).get("on_wait") or []
                if len(waits) > maxw:
                    for i in range(0, len(waits) - maxw, maxw):
                        uid[0] += 1
                        out.append({
                            "debug": ins.get("debug", 0),
                            "engine": ins["engine"],
                            "ins": [],
                            "name": f"{ins['name']}-w{uid[0]}",
                            "opcode": "NoOp",
                            "outs": [],
                            "sync_info": {
                                "on_update": [],
                                "on_wait": waits[i:i + maxw],
                            },
                        })
                    si["on_wait"] = waits[len(waits) - maxw:]
                out.append(ins)
            bb["instructions"] = out
    return m


def _install_wait_split(nc):
    import orjson
    orig = nc.to_json_bytes

    def patched():
        return orjson.dumps(_split_bir_waits(orjson.loads(orig())))

    nc.to_json_bytes = patched


def _build_program(Lslot):
    """SPMD Bass program for one core handling S=2 slots of Lslot
    (multiple of 256) padded kv tokens each."""
    from concourse import bass, mybir
    from concourse.tile import TileContext

    _patch_tile_drain()

    f32 = mybir.dt.float32
    bf16 = mybir.dt.bfloat16
    fp8 = mybir.dt.float8e4
    fp8e5 = mybir.dt.float8e5
    Exp = mybir.ActivationFunctionType.Exp
    DR = mybir.MatmulPerfMode.DoubleRow
    Mul = mybir.AluOpType.mult
    Add = mybir.AluOpType.add

    NB = Lslot // 128          # 128-token blocks per slot
    NIT = Lslot // 256         # 256-token iterations per slot
    NT = S * NB
    NCH = S * NIT              # kv chunks

    nc = bass.Bass()

    kv_d = nc.declare_dram_parameter("kv", [NCH, 128, 2, 256], fp8,
                                     isOutput=False)
    qh_d = nc.declare_dram_parameter("qh", [128, S * 2 * 800], fp8,
                                     isOutput=False)
    qres_d = nc.declare_dram_parameter("qres", [S * NQ, D], f32,
                                       isOutput=False)
    maskb_d = nc.declare_dram_parameter("maskb", [128, NT], f32,
                                        isOutput=False)
    wvx_d = nc.declare_dram_parameter("wvx", [128, 512], fp8, isOutput=False)
    woT_d = nc.declare_dram_parameter("woT", [128, 512], bf16, isOutput=False)
    ones_d = nc.declare_dram_parameter("ones2", [128, 2], fp8, isOutput=False)
    ident_d = nc.declare_dram_parameter("ident", [128, 128], bf16,
                                        isOutput=False)
    out_d = nc.declare_dram_parameter("out", [S * NQ, D], f32, isOutput=True)

    with TileContext(nc, trace_sim=TRACE_SIM) as tc:
        with tc.tile_pool(name="const", bufs=1) as cpool, \
             tc.tile_pool(name="vp", bufs=2, space="PSUM") as vpp, \
             tc.tile_pool(name="sp", bufs=2, space="PSUM") as spp, \
             tc.tile_pool(name="cx", bufs=1, space="PSUM") as cxp, \
             tc.tile_pool(name="kv", bufs=3) as kvp, \
             tc.tile_pool(name="vt", bufs=2) as vtp, \
             tc.tile_pool(name="pb", bufs=2) as pbp, \
             tc.tile_pool(name="fin", bufs=1, space="PSUM") as finp:

            # ---- constants / small tensors ----
            wvx_sb = cpool.tile([128, 512], fp8)
            woT_sb = cpool.tile([128, 512], bf16)
            qh_sb = cpool.tile([128, S * 1600], fp8)
            qres_sb = cpool.tile([128, S * D], f32)
            maskb_sb = cpool.tile([128, NT], f32)
            ones_sb = cpool.tile([128, 2], fp8)
            ident_sb = cpool.tile([128, 128], bf16)
            linv_sb = cpool.tile([128, S * 8], f32)
            dummy_sb = cpool.tile([1, 2], f32)
            ctxn_sb = cpool.tile([128, 256], bf16)
            ctxT_sb = cpool.tile([128, 256], bf16)
            out_sb = cpool.tile([128, S * D], f32)

            # warmup-critical loads go first on the sync queue: the first
            # iteration's kT is computed on the host and DMAed straight in,
            # so the first scores/exp wait only on maskb + qT + kT0, not on
            # the kv -> K-proj -> copy chain
            def emit_warmup_dmas():
                nc.sync.dma_start(out=maskb_sb[:], in_=maskb_d[:])
                nc.sync.dma_start(out=qh_sb[:, 0:800], in_=qh_d[:, 0:800])
                nc.sync.dma_start(out=qh_sb[:, 800:1600],
                                  in_=qh_d[:, 800:1600])

            # remaining parameters drip one per iteration between kv chunk
            # loads so no kv chunk queues behind them on the sync SEQ
            param_drip = [
                lambda: nc.sync.dma_start(out=qh_sb[:, 1600:3200],
                                          in_=qh_d[:, 1600:3200]),
                lambda: nc.sync.dma_start(out=woT_sb[:], in_=woT_d[:]),
                lambda: nc.sync.dma_start(out=ident_sb[:], in_=ident_d[:]),
                lambda: nc.sync.dma_start(
                    out=qres_sb[0:NQ, :].rearrange("n (s d) -> n s d", s=S),
                    in_=qres_d[:].rearrange("(s n) d -> n s d", s=S)),
            ]

            ones3 = ones_sb[:].rearrange("p (t j) -> p t j", t=2)

            def emit_ctx(ctx, p3, v3, it_c, split=False):
                # ctx[q, h*32+d] and l[q, h] accumulate over iterations;
                # all 16 regions share one PSUM bank = one zero region, so
                # only the very first matmul starts, only the very last
                # stops. split=True issues per-block matmuls (block 0's ctx
                # overlaps block 1's exp) — used for the very last iteration
                # to shorten the end-of-kernel tail.
                first = it_c == 0
                last = it_c == NIT - 1
                if not split:
                    for h in range(H):
                        g, hh = divmod(h, 4)
                        ph = p3[:, :, g * 400 + hh * 100:
                                g * 400 + hh * 100 + 100]
                        nc.tensor.matmul(
                            out=ctx[0:NQ, h * 32:(h + 1) * 32],
                            lhsT=ph,
                            rhs=v3[:, :, h * 32:(h + 1) * 32],
                            start=(first and h == 0), stop=False,
                            perf_mode=DR, skip_group_check=True)
                        nc.tensor.matmul(
                            out=ctx[0:NQ, 256 + h:257 + h],
                            lhsT=ph,
                            rhs=ones3,
                            start=False, stop=(last and h == H - 1),
                            perf_mode=DR, skip_group_check=True)
                    return
                for b in range(2):
                    for h in range(H):
                        g, hh = divmod(h, 4)
                        ph = p3[:, b, g * 400 + hh * 100:
                                g * 400 + hh * 100 + 100]
                        nc.tensor.matmul(
                            out=ctx[0:NQ, h * 32:(h + 1) * 32],
                            lhsT=ph,
                            rhs=v3[:, b, h * 32:(h + 1) * 32],
                            start=(first and b == 0 and h == 0), stop=False,
                            skip_group_check=True)
                        nc.tensor.matmul(
                            out=ctx[0:NQ, 256 + h:257 + h],
                            lhsT=ph,
                            rhs=ones_sb[:, b:b + 1],
                            start=False,
                            stop=(last and b == 1 and h == H - 1),
                            skip_group_check=True)

            def emit_finalize(ctx, s):
                nc.vector.reciprocal(
                    out=linv_sb[0:NQ, s * 8:(s + 1) * 8],
                    in_=ctx[0:NQ, 256:264])
                # ctx_norm = ctx * (1/l) / WSCALE (v-path scale; the k-path
                # scale already cancelled against q-tilde); exp bias cancels
                linv_b = linv_sb[0:NQ, s * 8:(s + 1) * 8][:, :, None] \
                    .broadcast_to([NQ, 8, 32])
                nc.vector.scalar_tensor_tensor(
                    out=ctxn_sb[0:NQ, :].rearrange("p (h d) -> p h d", h=8),
                    in0=ctx[0:NQ, 0:256].rearrange("p (h d) -> p h d", h=8),
                    scalar=1.0 / WSCALE,
                    in1=linv_b,
                    op0=Mul, op1=Mul)
                # transpose -> ctxT [d, q] for out-proj lhsT
                ctxT_ps = finp.tile([128, 1024], bf16, tag="fin",
                                    name=f"ct{s}")
                for kh in range(2):
                    nc.tensor.matmul(
                        out=ctxT_ps[:, kh * 100:(kh + 1) * 100],
                        lhsT=ctxn_sb[0:NQ, kh * 128:(kh + 1) * 128],
                        rhs=ident_sb[0:NQ, 0:NQ],
                        is_transpose=True,
                        start=(kh == 0), stop=(kh == 1))
                nc.vector.tensor_copy(ctxT_sb[:, 0:200], ctxT_ps[:, 0:200])
                # out-projection + residual (qres already holds
                # query + bv@Wo.T + bo)
                op_ps = finp.tile([128, 512], f32, tag="fin", name=f"op{s}")
                wo3 = woT_sb[:].rearrange("p (t j) -> p t j", t=2)
                for kh in range(2):
                    nc.tensor.matmul(
                        out=op_ps[0:NQ, 0:256],
                        lhsT=ctxT_sb[:, kh * 100:(kh + 1) * 100],
                        rhs=wo3[:, kh, :],
                        start=(kh == 0), stop=(kh == 1))
                # residual + store in halves so the first DMA overlaps the
                # second half's add (shortens the end-of-kernel tail)
                for hf in range(2):
                    nc.vector.tensor_tensor(
                        out=out_sb[0:NQ, s * 256 + hf * 128:
                                   s * 256 + (hf + 1) * 128],
                        in0=op_ps[0:NQ, hf * 128:(hf + 1) * 128],
                        in1=qres_sb[0:NQ, s * 256 + hf * 128:
                                    s * 256 + (hf + 1) * 128],
                        op=Add)
                    nc.scalar.dma_start(
                        out=out_d[s * NQ:(s + 1) * NQ,
                                  hf * 128:(hf + 1) * 128],
                        in_=out_sb[0:NQ, s * 256 + hf * 128:
                                   s * 256 + (hf + 1) * 128])

            # warm the Act engine's Exp table during DMA warmup so the
            # first real exp doesn't pay the 1.3us table load; wvx and ones
            # ride the scalar queue, which is otherwise idle until the
            # first exp becomes ready
            nc.gpsimd.memset(dummy_sb[:], 0.0)
            nc.scalar.activation(dummy_sb[0:1, 1:2], dummy_sb[0:1, 0:1], Exp)
            nc.scalar.dma_start(out=wvx_sb[:], in_=wvx_d[:])
            nc.scalar.dma_start(out=ones_sb[:], in_=ones_d[:])
            emit_warmup_dmas()

            fin_pend = None
            drip_i = [0]
            for s in range(S):
                ctx = cxp.tile([128, 512], f32, tag="cx", name=f"cx{s}")
                # software-pipelined: iteration it's ctx/l matmuls are
                # emitted after iteration it+1's scores, so the in-order PE
                # stream never stalls on the Act engine's exp; the previous
                # slot's finalize is likewise deferred into this slot's
                # first iteration
                pend = None
                for it in range(NIT):
                    ch = s * NIT + it

                    kv_sb = kvp.tile([128, 512], fp8, tag="kv")
                    nc.sync.dma_start(
                        out=kv_sb[:].rearrange("p (t m) -> p t m", t=2),
                        in_=kv_d[ch])
                    if drip_i[0] < len(param_drip) and (s > 0 or it > 0):
                        # flush all remaining on the last slot's first
                        # iteration (covers tiny NIT); else one per iteration
                        n = len(param_drip) if s == S - 1 else 1
                        for _ in range(n):
                            if drip_i[0] < len(param_drip):
                                param_drip[drip_i[0]]()
                                drip_i[0] += 1
                    kv3 = kv_sb[:].rearrange("p (t m) -> p t m", t=2)

                    # scores: kv^T @ (Wk^T q-tilde) with the K-projection
                    # folded into the host-precomputed qh (fp8, x8), one
                    # DoubleRow matmul per (block, head-group); exp applies
                    # the 1/8 via its scale and the mask via its bias
                    p_sb = pbp.tile([128, 1600], fp8e5, tag="pb")
                    p3 = p_sb[:].rearrange("p (t c) -> p t c", t=2)
                    for b in range(2):
                        blk = s * NB + it * 2 + b
                        sp = spp.tile([128, 1024], f32, tag="sp")
                        sp3 = sp[:].rearrange("p (g c) -> p g c", g=2)
                        for g in range(2):
                            qh3 = qh_sb[:, (s * 2 + g) * 800:
                                        (s * 2 + g + 1) * 800].rearrange(
                                "p (k c) -> p k c", k=2)
                            nc.tensor.matmul(
                                out=sp[:, g * 512:g * 512 + 400],
                                lhsT=kv3[:, :, b * 128:(b + 1) * 128],
                                rhs=qh3,
                                start=True, stop=True, perf_mode=DR)
                        nc.scalar.activation(
                            p3[:, b, :], sp3[:, :, 0:400], Exp,
                            bias=maskb_sb[:, blk:blk + 1], scale=0.125)

                    # V-proj: v[b*256 + dout] natural, DoubleRow K=256
                    vp = vpp.tile([128, 512], f32, tag="vp")
                    wv3 = wvx_sb[:].rearrange("p (t j) -> p t j", t=2)
                    for b in range(2):
                        nc.tensor.matmul(
                            out=vp[:, b * 256:(b + 1) * 256],
                            lhsT=kv3[:, :, b * 128:(b + 1) * 128],
                            rhs=wv3,
                            start=(b == 0), stop=(b == 1), perf_mode=DR)
                    v_sb = vtp.tile([128, 512], fp8, tag="vt")
                    nc.vector.tensor_copy(v_sb[:], vp[:])
                    v3 = v_sb[:].rearrange("p (t j) -> p t j", t=2)

                    if pend is not None:
                        emit_ctx(ctx, *pend)
                    elif fin_pend is not None:
                        emit_finalize(*fin_pend)
                        fin_pend = None
                    pend = (p3, v3, it)

                emit_ctx(ctx, *pend, split=(s == S - 1))
                if fin_pend is not None:
                    # NIT == 1: previous slot's finalize still pending
                    emit_finalize(*fin_pend)
                fin_pend = (ctx, s)

            emit_finalize(*fin_pend)

    _install_wait_split(nc)
    return nc


def _get_program(Lslot):
    if Lslot not in _prog_cache:
        _prog_cache[Lslot] = _build_program(Lslot)
    return _prog_cache[Lslot]


def kernel(source, query, batch_offsets, Wq, bq, Wk, bk, Wv, bv, Wo, bo):
    from concourse.bass_utils import run_bass_kernel_spmd

    source = np.asarray(source, dtype=np.float32)
    query = np.asarray(query, dtype=np.float32)
    offs = np.asarray(batch_offsets).astype(np.int64)
    Wq = np.asarray(Wq, np.float32); bq = np.asarray(bq, np.float32)
    Wk = np.asarray(Wk, np.float32); bk = np.asarray(bk, np.float32)
    Wv = np.asarray(Wv, np.float32); bv = np.asarray(bv, np.float32)
    Wo = np.asarray(Wo, np.float32); bo = np.asarray(bo, np.float32)
    B = query.shape[0]
    assert B == NCORES * S

    lens = offs[1:] - offs[:-1]
    Lmax = int(lens.max()) if len(lens) else 1
    Lslot = max(256, _ceil_to(max(Lmax, 1), 256))
    NB = Lslot // 128
    NIT = Lslot // 256
    NT = S * NB

    nc = _get_program(Lslot)

    scale = 1.0 / np.sqrt(np.float32(HD))

    # Shared (replicated) weight packs.
    wv_s = (Wv * WSCALE).astype(np.float32)
    wvx = np.empty((128, 2, 256), np.float32)
    for kh in range(2):
        # Wv.T chunk: [din 128, dout 256]
        wvx[:, kh, :] = wv_s.T[kh * 128:(kh + 1) * 128, :]
    wvx = wvx.reshape(128, 512).astype(FP8)
    woT = np.empty((128, 2, 256), np.float32)
    for kh in range(2):
        woT[:, kh, :] = Wo.T[kh * 128:(kh + 1) * 128, :]
    woT = woT.reshape(128, 512).astype(BF16)
    ones2 = np.ones((128, 2), FP8)
    ident = np.eye(128, dtype=np.float32).astype(BF16)

    # q-tilde at true scale: (query @ Wq.T + bq) * scale
    qt_all = ((query.reshape(B * NQ, D) @ Wq.T + bq) * scale)
    qt_all = qt_all.reshape(B, NQ, H, HD)

    # residual with folded bv/bo: query + bv @ Wo.T + bo
    resid_bias = (bv @ Wo.T + bo).astype(np.float32)

    in_maps = []
    for c in range(NCORES):
        kv = np.zeros((S * NIT, 128, 2, 256), np.float32)
        maskb = np.full((128, NT), -1e30, np.float32)
        qh = np.zeros((128, S * 2, 2, 400), np.float32)
        for s in range(S):
            bidx = c * S + s
            L = int(lens[bidx])
            if L > 0:
                seg = source[offs[bidx]:offs[bidx] + L]  # [L, D]
                segT = seg.T  # [D, L]
                # chunk ch=(s*NIT+it) holds tokens [it*256,(it+1)*256):
                # kv[ch, p, kh, m] = source[tok, kh*128+p]
                nfull_it = L // 256
                for it in range(nfull_it + (1 if L % 256 else 0)):
                    t0 = it * 256
                    t1 = min(L, t0 + 256)
                    blkT = segT[:, t0:t1]  # [256 din, tk]
                    kv[s * NIT + it, :, :, 0:t1 - t0] = (
                        blkT.reshape(2, 128, t1 - t0).transpose(1, 0, 2))
                nfull = L // 128
                maskb[:, s * NB: s * NB + nfull] = EXPBIAS
                if L % 128:
                    maskb[0:L % 128, s * NB + nfull] = EXPBIAS
            else:
                # empty segment: expose one zero token so l stays finite;
                # these rows are recomputed exactly on the host below
                maskb[0, s * NB] = EXPBIAS
            # block-diag q-tilde, then fold the K-projection into it:
            # qh[(s,g)][din, hq] = 8 * sum_dout Wk[dout, din] qTz[dout, hq]
            for g in range(2):
                qTz = np.zeros((128, 400), np.float32)
                for hh in range(4):
                    qTz[hh * 32:(hh + 1) * 32, hh * 100:hh * 100 + NQ] = \
                        qt_all[bidx, :, g * 4 + hh, :].T
                qh_g = 8.0 * (Wk[g * 128:(g + 1) * 128, :].T @ qTz)
                qh[:, (s * 2 + g), 0, :] = qh_g[0:128, :]
                qh[:, (s * 2 + g), 1, :] = qh_g[128:256, :]
        q2 = query[c * S:(c + 1) * S].reshape(S * NQ, D)
        qres = np.ascontiguousarray(q2 + resid_bias[None, :])
        kv8 = kv.astype(FP8)
        in_maps.append({
            "kv": kv8,
            "qh": qh.reshape(128, S * 1600).astype(FP8),
            "qres": qres, "maskb": maskb,
            "wvx": wvx, "woT": woT,
            "ones2": ones2, "ident": ident,
        })

    res = run_bass_kernel_spmd(nc, in_maps, list(range(NCORES)))
    out = np.concatenate(
        [res.results[c]["out"].reshape(S, NQ, D) for c in range(NCORES)],
        axis=0).astype(np.float32)

    # Empty segments: reference attends uniformly over Lmax copies of
    # source[0] -> ctx = v(source[0]); compute exactly on host.
    for bidx in range(B):
        if lens[bidx] == 0:
            v0 = source[0] @ Wv.T + bv
            out[bidx] = (v0 @ Wo.T + bo)[None, :] + query[bidx]

    return out


if __name__ == "__main__":
    pass
